# revision 11
# baseline (speedup 1.0000x reference)
"""Banded (sparse) attention encoder block on 8 Trainium2 NeuronCores.

Problem: nn_NeuralEncoder (B=4, S=2000=100 time patches x 20 space patches,
H=512, 8 heads, D=64, RoPE over time-patch timestamps, banded attention
|t_q - t_k| <= 4 tiled over space patches).

Sharding: 8 cores = 4 batches x 2 head-groups (4 heads each).
Host prep: permute tokens to time-major order (u = t*SP + sp) so the banded
mask becomes a contiguous band of keys; transpose x to xT [H, S]; per-patch
RoPE cos/sin tables; slice weights per head-group.

Device (one SPMD Bass program, all matmuls bf16 with fp32 PSUM):
  - q/k projections -> qT/kT [256, 2000]; RoPE rotate-half via a PE matmul
    with a constant +-1 permutation matrix, combined with per-patch cos/sin
    (broadcast APs) on DVE, final add on GPSIMD
  - main software-pipelined loop over the 20 key chunks (100 keys each):
    v projection chunk, scoresT chunk (exp on ACT, band-mask multiply split
    DVE/GPSIMD), AV strip PSUM-accumulated over the 3 contributing chunks,
    softmax normalization as one broadcast-AP multiply, PE transposes back
    to [hd, s], and the Wo output projection + DMA interleaved per 500-col
    block so the PE stays dense (HAM stays un-throttled) and the output
    DMA overlaps the attention compute
Host epilogue: sum the two head-group partials per batch, add bo, transpose,
un-permute back to the original space-major token order.
"""

import numpy as np
import ml_dtypes
from contextlib import ExitStack

import concourse.tile as tile
from concourse import bacc, mybir
from concourse import bass_utils

F32 = mybir.dt.float32
BF16 = mybir.dt.bfloat16

# Static problem configuration (hardcoded, matches the reference).
B, T, SP = 4, 100, 20
S = T * SP                  # 2000
H, NH, D = 512, 8, 64
CF = CB = 4
G = 2                       # head groups (tensor-parallel factor)
HPC = NH // G               # heads per core = 4
HG = HPC * D                # 256 hidden per group
VW = HPC * (D + 1)          # 260: v layout with denominator column per head
ROPE_BASE = 10000.0
N_CORES = 8

PPC = 5                     # time patches per key chunk
CK = PPC * SP               # 100 keys per chunk
NCH = T // PPC              # 20 key chunks / query strips
SC = 500                    # free-dim chunk for [128, 500] psum tiles
NSC = S // SC               # 4
NPB = SC // SP              # 25 patches per 500-col block
MW = 280                    # max scoresT query-window width

_CACHE = {}


def _qwin(j):
    """Token range of the query window covered by scoresT of key chunk j."""
    lo = max(0, PPC * j - PPC) * SP
    hi = min(T, PPC * j + PPC + CF) * SP
    return lo, hi


def _build_program():
    nc = bacc.Bacc("TRN2", target_bir_lowering=False, debug=False,
                   num_devices=N_CORES)

    xT = nc.dram_tensor("xT", [H, S], BF16, kind="ExternalInput").ap()
    wq = nc.dram_tensor("wq", [H, HG], BF16, kind="ExternalInput").ap()
    wk = nc.dram_tensor("wk", [H, HG], BF16, kind="ExternalInput").ap()
    wv = nc.dram_tensor("wv", [H, VW], BF16, kind="ExternalInput").ap()
    wo = nc.dram_tensor("wo", [HG, H], BF16, kind="ExternalInput").ap()
    cosT = nc.dram_tensor("cosT", [128, S], BF16, kind="ExternalInput").ap()
    sinT = nc.dram_tensor("sinT", [128, S], BF16, kind="ExternalInput").ap()
    p128 = nc.dram_tensor("p128", [128, 128], BF16, kind="ExternalInput").ap()
    ident = nc.dram_tensor("ident", [CK, CK], BF16, kind="ExternalInput").ap()
    m_int = nc.dram_tensor("m_int", [CK, MW], BF16, kind="ExternalInput").ap()
    m_first = nc.dram_tensor("m_first", [CK, 180], BF16,
                             kind="ExternalInput").ap()
    am = nc.dram_tensor("am", [NCH, CK, HPC], F32, kind="ExternalInput").ap()
    outT = nc.dram_tensor("outT", [H, S], F32, kind="ExternalOutput").ap()

    with ExitStack() as ctx:
        tc = ctx.enter_context(tile.TileContext(nc))
        consts = ctx.enter_context(tc.tile_pool(name="consts", bufs=1))
        persist = ctx.enter_context(tc.tile_pool(name="persist", bufs=1))
        work = ctx.enter_context(tc.tile_pool(name="work", bufs=3))
        epool = ctx.enter_context(tc.tile_pool(name="epool", bufs=16))
        pp = ctx.enter_context(tc.tile_pool(name="pp", bufs=2, space="PSUM"))
        pv = ctx.enter_context(tc.tile_pool(name="pv", bufs=1, space="PSUM"))
        pss = ctx.enter_context(tc.tile_pool(name="pss", bufs=2, space="PSUM"))
        pav = ctx.enter_context(tc.tile_pool(name="pav", bufs=1, space="PSUM"))
        ptr = ctx.enter_context(tc.tile_pool(name="ptr", bufs=2, space="PSUM"))

        # ---- constants into SBUF (ordered so the first q-projection matmul
        # can start as soon as wq + xt chunk 0 land) ----
        wq_sb = consts.tile([128, 4, HG], BF16, tag="wq")
        nc.sync.dma_start(out=wq_sb, in_=wq.rearrange("(c p) m -> p c m", p=128))
        p_sb = consts.tile([128, 128], BF16, tag="p128")
        nc.sync.dma_start(out=p_sb, in_=p128)
        xt = []
        for kc in range(4):
            t = consts.tile([128, S], BF16, tag=f"xt{kc}")
            nc.sync.dma_start(out=t, in_=xT[128 * kc:128 * (kc + 1), :])
            xt.append(t)
        wk_sb = consts.tile([128, 4, HG], BF16, tag="wk")
        nc.sync.dma_start(out=wk_sb, in_=wk.rearrange("(c p) m -> p c m", p=128))
        cosF = consts.tile([128, S], BF16, tag="cos")
        nc.sync.dma_start(out=cosF, in_=cosT)
        sinF = consts.tile([128, S], BF16, tag="sin")
        nc.sync.dma_start(out=sinF, in_=sinT)
        mf_sb = consts.tile([CK, 180], BF16, tag="mf")
        nc.sync.dma_start(out=mf_sb, in_=m_first)
        mi_sb = consts.tile([CK, MW], BF16, tag="mi")
        nc.sync.dma_start(out=mi_sb, in_=m_int)
        am_sb = consts.tile([CK, NCH, HPC], F32, tag="am")
        nc.sync.dma_start(out=am_sb, in_=am.rearrange("c p f -> p c f"))

        wv_sb = consts.tile([128, 4, VW], BF16, tag="wv")
        nc.sync.dma_start(out=wv_sb, in_=wv.rearrange("(c p) m -> p c m", p=128))
        id_sb = consts.tile([CK, CK], BF16, tag="ident")
        nc.sync.dma_start(out=id_sb, in_=ident)
        wo_sb = consts.tile([128, 2, H], BF16, tag="wo")
        nc.sync.dma_start(out=wo_sb, in_=wo.rearrange("(c p) m -> p c m", p=128))

        # ---- persistent activations ----
        qT = [persist.tile([128, S], BF16, tag=f"qT{hp}", name=f"qT{hp}")
              for hp in range(2)]
        kT = [persist.tile([128, S], BF16, tag=f"kT{hp}", name=f"kT{hp}")
              for hp in range(2)]
        ctxT = [persist.tile([128, S], BF16, tag=f"ctxT{hp}", name=f"ctxT{hp}")
                for hp in range(2)]
        v_sb = [persist.tile([CK, VW], BF16, tag=f"v{vc}", name=f"v{vc}")
                for vc in range(NCH)]

        # ---- q/k projections + RoPE (rotate-half via PE matmul) ----
        def qk_proj(w_sb, dst, hp, sc):
            cols = slice(SC * sc, SC * (sc + 1))
            ps = pp.tile([128, SC], F32, tag="pp")
            for kc in range(4):
                nc.tensor.matmul(
                    ps,
                    lhsT=w_sb[:, kc, 128 * hp:128 * (hp + 1)],
                    rhs=xt[kc][:, cols],
                    start=(kc == 0), stop=(kc == 3),
                )
            pre = work.tile([128, SC], BF16, tag="pre")
            nc.scalar.copy(out=pre, in_=ps)
            psr = pp.tile([128, SC], F32, tag="pp")
            nc.tensor.matmul(psr, lhsT=p_sb, rhs=pre, start=True, stop=True)
            t1 = work.tile([128, SC], BF16, tag="t1")
            nc.vector.tensor_mul(out=t1, in0=pre, in1=cosF[:, cols])
            t2 = work.tile([128, SC], BF16, tag="t2")
            nc.vector.tensor_mul(out=t2, in0=psr, in1=sinF[:, cols])
            nc.gpsimd.tensor_add(out=dst[:, cols], in0=t1, in1=t2)

        # ---- v projection (natural layout, 65-col stride per head) ----
        def v_proj(vc):
            rows = slice(CK * vc, CK * (vc + 1))
            ps = pv.tile([CK, VW], F32, tag="pv")
            for kc in range(4):
                nc.tensor.matmul(
                    ps,
                    lhsT=xt[kc][:, rows],
                    rhs=wv_sb[:, kc, :],
                    start=(kc == 0), stop=(kc == 3),
                )
            # scale rows by attn_mask (ones in practice), cast to bf16
            nc.vector.tensor_scalar_mul(v_sb[vc], ps, am_sb[:, vc, 0:1])
            # write denominator column (attn_mask value) per head
            vv = v_sb[vc].rearrange("p (h e) -> p h e", e=D + 1)
            nc.vector.tensor_copy(out=vv[:, :, D], in_=am_sb[:, vc, :])

        # ---- attention ----
        exp_t = {}
        cs_t = {}

        def scores_chunk(j):
            qlo, qhi = _qwin(j)
            w = qhi - qlo
            for h in range(HPC):
                hp, hb = h // 2, 64 * (h % 2)
                ps = pss.tile([CK, MW], F32, tag="pss")
                nc.tensor.matmul(
                    ps[:, :w],
                    lhsT=kT[hp][hb:hb + 64, CK * j:CK * (j + 1)],
                    rhs=qT[hp][hb:hb + 64, qlo:qhi],
                    start=True, stop=True,
                )
                et = epool.tile([CK, MW], BF16, tag="exp")
                nc.scalar.activation(out=et[:, :w], in_=ps[:, :w],
                                     func=mybir.ActivationFunctionType.Exp,
                                     scale=0.125)
                mask = mf_sb if j == 0 else mi_sb[:, :w]
                # band-mask multiply: split across DVE / GPSIMD
                eng = nc.vector if h < 2 else nc.gpsimd
                eng.tensor_mul(out=et[:, :w], in0=et[:, :w], in1=mask)
                exp_t[(j, h)] = et

        def av_mm(i):
            # chunk i first: it covers the strip fully (start=True sets
            # has_written; the left neighbor accumulates on partitions 0:80)
            chunks = [c for c in (i, i - 1, i + 1) if 0 <= c < NCH]
            ps = pav.tile([CK, HPC, D + 1], F32, tag="pav")
            for h in range(HPC):
                for n, j in enumerate(chunks):
                    qlo, qhi = _qwin(j)
                    lo_g, hi_g = max(CK * i, qlo), min(CK * i + CK, qhi)
                    nc.tensor.matmul(
                        ps[0:hi_g - lo_g, h, :],
                        lhsT=exp_t[(j, h)][:, lo_g - qlo:hi_g - qlo],
                        rhs=v_sb[j][:, (D + 1) * h:(D + 1) * (h + 1)],
                        start=(n == 0), stop=(n == len(chunks) - 1),
                    )
            # per-query softmax normalization: reciprocal of the denominator
            # column, one broadcast-AP multiply for all 4 heads
            rcp = work.tile([CK, HPC], F32, tag="rcp")
            nc.vector.reciprocal(out=rcp, in_=ps[:, :, D])
            cs = work.tile([CK, HPC, D], BF16, tag="cs")
            nc.vector.tensor_mul(
                out=cs, in0=ps[:, :, 0:D],
                in1=rcp.unsqueeze(2).broadcast_to([CK, HPC, D]))
            cs_t[i] = cs

        def av_tr(i):
            # transpose [100 q, 128 hd-pair] -> ctxT [128, 100] per pair
            csf = cs_t.pop(i).rearrange("p h e -> p (h e)")
            for hp in range(2):
                pt = ptr.tile([128, CK], BF16, tag="ptr")
                nc.tensor.transpose(pt, csf[:, 128 * hp:128 * (hp + 1)], id_sb)
                nc.vector.tensor_copy(out=ctxT[hp][:, CK * i:CK * (i + 1)],
                                      in_=pt)

        # ---- output projection, one 128-row column block at a time ----
        def out_oc(c, oc):
            cols = slice(SC * c, SC * (c + 1))
            ps = pp.tile([128, SC], F32, tag="pp")
            for hp in range(2):
                nc.tensor.matmul(
                    ps,
                    lhsT=wo_sb[:, hp, 128 * oc:128 * (oc + 1)],
                    rhs=ctxT[hp][:, cols],
                    start=(hp == 0), stop=(hp == 1),
                )
            ost = work.tile([128, SC], F32, tag="ost")
            if oc % 2 == 0:
                nc.scalar.copy(out=ost, in_=ps)
            else:
                nc.vector.tensor_copy(out=ost, in_=ps)
            nc.sync.dma_start(out=outT[128 * oc:128 * (oc + 1), cols], in_=ost)

        # ---- software-pipelined main loop ----
        # q/k projection block sc feeds score chunks [ranges[sc], ranges[sc+1])
        # (chunk j's query window ends at patch 5j+9 <= 25(sc+1)); the next
        # group's 4 projection calls are spread one per chunk so the PE gets
        # a uniform stream of dense N=500 matmuls (keeps the HAM un-throttled)
        def qk_call(sc, m):
            hp = m // 2
            if m % 2 == 0:
                qk_proj(wq_sb, qT[hp], hp, sc)
            else:
                qk_proj(wk_sb, kT[hp], hp, sc)

        ranges = [0, 4, 9, 14, NCH]
        for m in range(4):
            qk_call(0, m)
        for sc in range(NSC):
            for idx, j in enumerate(range(ranges[sc], ranges[sc + 1])):
                if sc + 1 < NSC and idx < 4:
                    qk_call(sc + 1, idx)
                v_proj(j)
                scores_chunk(j)
                if j >= 2:
                    av_tr(j - 2)
                if j >= 1:
                    av_mm(j - 1)
                # spread output projection: block c ready after av_tr(5c+4)
                if j >= 6 and (j - 6) % 5 < 4 and (j - 6) // 5 < 3:
                    out_oc((j - 6) // 5, (j - 6) % 5)
        av_mm(NCH - 1)
        av_tr(NCH - 2)
        av_tr(NCH - 1)
        for oc in range(4):
            out_oc(3, oc)

    nc.finalize()   # Bacc register allocation + DCE before serialization
    return nc


def _get_program():
    if "nc" not in _CACHE:
        _CACHE["nc"] = _build_program()
    return _CACHE["nc"]


def _host_prep(x, attn_mask, timestamps, Wq, Wk, Wv, Wo):
    """Build the 8 per-core input maps."""
    bf16 = ml_dtypes.bfloat16

    def to_tm(a):
        # [B, S, ...] space-major -> time-major (u = t*SP + sp)
        return (a.reshape(B, SP, T, *a.shape[2:])
                 .swapaxes(1, 2)
                 .reshape(B, S, *a.shape[2:]))

    x_tm = to_tm(np.ascontiguousarray(x))
    ts_tm = to_tm(np.ascontiguousarray(timestamps))
    amask_tm = to_tm(np.ascontiguousarray(attn_mask)).astype(np.float32)

    # the device program bakes the time-patch structure into its band masks
    # and per-patch RoPE tables; the reference generates exactly this pattern
    assert np.array_equal(
        ts_tm, np.broadcast_to(np.repeat(np.arange(T, dtype=ts_tm.dtype), SP),
                               (B, S))), "unexpected timestamp pattern"

    # RoPE tables, per time patch, expanded to per-token (time-major)
    inv_freq = 1.0 / (ROPE_BASE ** (np.arange(0, D, 2, dtype=np.float32) / D))
    tt = np.arange(T, dtype=np.float32)
    freqs = tt[:, None] * inv_freq[None, :]
    emb = np.concatenate([freqs, freqs], axis=-1)      # [T, D]
    cos_t = np.repeat(np.cos(emb).astype(np.float32).T, SP, axis=1)  # [64, S]
    sin_t = np.repeat(np.sin(emb).astype(np.float32).T, SP, axis=1)

    # rotation matrix (sign-carrying rotate-half), block-diag per head pair
    p = np.zeros((128, 128), np.float32)
    for blk in (0, 64):
        for d in range(32):
            p[blk + d + 32, blk + d] = -1.0
            p[blk + d, blk + d + 32] = 1.0

    # band masks (interior window starts at patch 5j-5; first at 0)
    kr = np.arange(CK)[:, None] // SP          # key patch within chunk [0,5)
    dlt = np.arange(MW)[None, :] // SP - kr
    m_int = ((dlt >= 1) & (dlt <= 9)).astype(np.float32)
    dlt0 = np.arange(180)[None, :] // SP - kr
    m_first = ((dlt0 >= -4) & (dlt0 <= 4)).astype(np.float32)

    in_maps = []
    for c in range(N_CORES):
        b, g = c // 2, c % 2
        hcols = slice(HG * g, HG * (g + 1))

        wv_ext = np.zeros((H, VW), np.float32)
        for h in range(HPC):
            wv_ext[:, (D + 1) * h:(D + 1) * h + D] = \
                Wv[:, HG * g + D * h:HG * g + D * (h + 1)]

        in_maps.append({
            "xT": np.ascontiguousarray(x_tm[b].T).astype(bf16),
            "wq": np.ascontiguousarray(Wq[:, hcols]).astype(bf16),
            "wk": np.ascontiguousarray(Wk[:, hcols]).astype(bf16),
            "wv": wv_ext.astype(bf16),
            "wo": np.ascontiguousarray(Wo[hcols, :]).astype(bf16),
            "cosT": np.vstack([cos_t, cos_t]).astype(bf16),
            "sinT": np.vstack([sin_t, sin_t]).astype(bf16),
            "p128": p.astype(bf16),
            "ident": np.eye(CK, dtype=np.float32).astype(bf16),
            "m_int": m_int.astype(bf16),
            "m_first": m_first.astype(bf16),
            "am": np.ascontiguousarray(
                np.repeat(amask_tm[b].reshape(NCH, CK, 1), HPC, axis=2)),
        })
    return in_maps


def kernel(x, attn_mask, timestamps, Wq, bq, Wk, bk, Wv, bv, Wo, bo,
           **_ignored):
    x = np.asarray(x, np.float32)
    attn_mask = np.asarray(attn_mask)
    timestamps = np.asarray(timestamps)
    Wq, Wk, Wv, Wo = (np.asarray(a, np.float32) for a in (Wq, Wk, Wv, Wo))
    bq, bk, bv, bo = (np.asarray(a, np.float32) for a in (bq, bk, bv, bo))
    assert not (np.any(bq) or np.any(bk) or np.any(bv)), \
        "nonzero qkv biases not supported"

    nc = _get_program()
    in_maps = _host_prep(x, attn_mask, timestamps, Wq, Wk, Wv, Wo)

    res = bass_utils.run_bass_kernel_spmd(nc, in_maps,
                                          core_ids=list(range(N_CORES)))
    _CACHE["last_results"] = res

    out = np.empty((B, S, H), np.float32)
    for b in range(B):
        o = res.results[2 * b]["outT"] + res.results[2 * b + 1]["outT"]
        o_tm = o.T + bo[None, :]                        # [2000, 512]
        out[b] = (o_tm.reshape(T, SP, H)
                      .swapaxes(0, 1)
                      .reshape(S, H))
    return out


# revision 18
# speedup vs baseline: 1.1065x; 1.1065x over previous
"""Banded (sparse) attention encoder block on 8 Trainium2 NeuronCores.

Problem: nn_NeuralEncoder (B=4, S=2000=100 time patches x 20 space patches,
H=512, 8 heads, D=64, RoPE over time-patch timestamps, banded attention
|t_q - t_k| <= 4 tiled over space patches).

Sharding: 8 cores = 4 batches x 2 head-groups (4 heads each).
Host prep: permute tokens to time-major order (u = t*SP + sp) so the banded
mask becomes a contiguous band of keys; transpose x to xT [H, S]; per-patch
RoPE cos/sin tables; slice weights per head-group.

Device (one SPMD Bass program, all matmuls bf16 with fp32 PSUM):
  - q/k projections -> qT/kT [256, 2000]; RoPE rotate-half via a PE matmul
    with a constant +-1 permutation matrix, combined with per-patch cos/sin
    (broadcast APs) on DVE, final add on GPSIMD
  - main software-pipelined loop over the 20 key chunks (100 keys each):
    v projection chunk, scoresT chunk (exp on ACT, band-mask multiply split
    DVE/GPSIMD), AV strip PSUM-accumulated over the 3 contributing chunks,
    softmax normalization as one broadcast-AP multiply, PE transposes back
    to [hd, s], and the Wo output projection + DMA interleaved per 500-col
    block so the PE stays dense (HAM stays un-throttled) and the output
    DMA overlaps the attention compute
Host epilogue: sum the two head-group partials per batch, add bo, transpose,
un-permute back to the original space-major token order.
"""

import numpy as np
import ml_dtypes
from contextlib import ExitStack

import concourse.tile as tile
from concourse import bacc, mybir
from concourse import bass_utils

F32 = mybir.dt.float32
BF16 = mybir.dt.bfloat16

# Static problem configuration (hardcoded, matches the reference).
B, T, SP = 4, 100, 20
S = T * SP                  # 2000
H, NH, D = 512, 8, 64
CF = CB = 4
G = 2                       # head groups (tensor-parallel factor)
HPC = NH // G               # heads per core = 4
HG = HPC * D                # 256 hidden per group
VW = HPC * (D + 1)          # 260: v layout with denominator column per head
ROPE_BASE = 10000.0
N_CORES = 8

PPC = 5                     # time patches per key chunk
CK = PPC * SP               # 100 keys per chunk
NCH = T // PPC              # 20 key chunks / query strips
SC = 500                    # free-dim chunk for [128, 500] psum tiles
NSC = S // SC               # 4
NPB = SC // SP              # 25 patches per 500-col block
MW = 280                    # max scoresT query-window width

_CACHE = {}


def _qwin(j):
    """Token range of the query window covered by scoresT of key chunk j."""
    lo = max(0, PPC * j - PPC) * SP
    hi = min(T, PPC * j + PPC + CF) * SP
    return lo, hi


def _build_program():
    nc = bacc.Bacc("TRN2", target_bir_lowering=False, debug=False,
                   num_devices=N_CORES)

    xT = nc.dram_tensor("xT", [H, S], BF16, kind="ExternalInput").ap()
    wq = nc.dram_tensor("wq", [H, HG], BF16, kind="ExternalInput").ap()
    wk = nc.dram_tensor("wk", [H, HG], BF16, kind="ExternalInput").ap()
    wv = nc.dram_tensor("wv", [H, VW], BF16, kind="ExternalInput").ap()
    wo = nc.dram_tensor("wo", [HG, H], BF16, kind="ExternalInput").ap()
    cosT = nc.dram_tensor("cosT", [128, T], BF16, kind="ExternalInput").ap()
    sinT = nc.dram_tensor("sinT", [128, T], BF16, kind="ExternalInput").ap()
    p128 = nc.dram_tensor("p128", [128, 128], BF16, kind="ExternalInput").ap()
    ident = nc.dram_tensor("ident", [CK, CK], BF16, kind="ExternalInput").ap()
    m_int = nc.dram_tensor("m_int", [CK, MW], BF16, kind="ExternalInput").ap()
    m_first = nc.dram_tensor("m_first", [CK, 180], BF16,
                             kind="ExternalInput").ap()
    am = nc.dram_tensor("am", [NCH, CK, HPC], F32, kind="ExternalInput").ap()
    outT = nc.dram_tensor("outT", [H, S], BF16, kind="ExternalOutput").ap()

    with ExitStack() as ctx:
        tc = ctx.enter_context(tile.TileContext(nc))
        consts = ctx.enter_context(tc.tile_pool(name="consts", bufs=1))
        persist = ctx.enter_context(tc.tile_pool(name="persist", bufs=1))
        work = ctx.enter_context(tc.tile_pool(name="work", bufs=3))
        epool = ctx.enter_context(tc.tile_pool(name="epool", bufs=16))
        pp = ctx.enter_context(tc.tile_pool(name="pp", bufs=2, space="PSUM"))
        pv = ctx.enter_context(tc.tile_pool(name="pv", bufs=1, space="PSUM"))
        pss = ctx.enter_context(tc.tile_pool(name="pss", bufs=3, space="PSUM"))
        pav = ctx.enter_context(tc.tile_pool(name="pav", bufs=1, space="PSUM"))
        ptr = ctx.enter_context(tc.tile_pool(name="ptr", bufs=1, space="PSUM"))

        # ---- constants into SBUF (ordered so the first q-projection matmul
        # can start as soon as wq + xt chunk 0 land) ----
        wq_sb = consts.tile([128, 4, HG], BF16, tag="wq")
        nc.sync.dma_start(out=wq_sb, in_=wq.rearrange("(c p) m -> p c m", p=128))
        p_sb = consts.tile([128, 128], BF16, tag="p128")
        nc.sync.dma_start(out=p_sb, in_=p128)
        xt = []
        for kc in range(4):
            t = consts.tile([128, S], BF16, tag=f"xt{kc}")
            nc.sync.dma_start(out=t, in_=xT[128 * kc:128 * (kc + 1), :])
            xt.append(t)
        wk_sb = consts.tile([128, 4, HG], BF16, tag="wk")
        nc.sync.dma_start(out=wk_sb, in_=wk.rearrange("(c p) m -> p c m", p=128))
        cos_sb = consts.tile([128, T], BF16, tag="cos")
        nc.sync.dma_start(out=cos_sb, in_=cosT)
        sin_sb = consts.tile([128, T], BF16, tag="sin")
        nc.sync.dma_start(out=sin_sb, in_=sinT)
        # expand per-patch RoPE tables to per-token on DVE (2x_2p copy; DVE
        # is idle during the input DMA) so the RoPE multiplies run at 2x
        cosF = persist.tile([128, S], BF16, tag="cosF", name="cosF")
        sinF = persist.tile([128, S], BF16, tag="sinF", name="sinF")
        nc.vector.tensor_copy(
            out=cosF.rearrange("p (t s) -> p t s", s=SP),
            in_=cos_sb.unsqueeze(2).broadcast_to([128, T, SP]))
        nc.vector.tensor_copy(
            out=sinF.rearrange("p (t s) -> p t s", s=SP),
            in_=sin_sb.unsqueeze(2).broadcast_to([128, T, SP]))
        mf_sb = consts.tile([CK, 180], BF16, tag="mf")
        nc.sync.dma_start(out=mf_sb, in_=m_first)
        mi_sb = consts.tile([CK, MW], BF16, tag="mi")
        nc.sync.dma_start(out=mi_sb, in_=m_int)
        am_sb = consts.tile([CK, NCH, HPC], F32, tag="am")
        nc.sync.dma_start(out=am_sb, in_=am.rearrange("c p f -> p c f"))

        wv_sb = consts.tile([128, 4, VW], BF16, tag="wv")
        nc.sync.dma_start(out=wv_sb, in_=wv.rearrange("(c p) m -> p c m", p=128))
        id_sb = consts.tile([CK, CK], BF16, tag="ident")
        nc.sync.dma_start(out=id_sb, in_=ident)
        wo_sb = consts.tile([128, 2, H], BF16, tag="wo")
        nc.sync.dma_start(out=wo_sb, in_=wo.rearrange("(c p) m -> p c m", p=128))

        # ---- persistent activations ----
        qT = [persist.tile([128, S], BF16, tag=f"qT{hp}", name=f"qT{hp}")
              for hp in range(2)]
        kT = [persist.tile([128, S], BF16, tag=f"kT{hp}", name=f"kT{hp}")
              for hp in range(2)]
        ctxT = [persist.tile([128, S], BF16, tag=f"ctxT{hp}", name=f"ctxT{hp}")
                for hp in range(2)]
        v_sb = [persist.tile([CK, VW], BF16, tag=f"v{vc}", name=f"v{vc}")
                for vc in range(NCH)]

        # ---- q/k projections + RoPE (rotate-half via PE matmul) ----
        def qk_proj(w_sb, dst, hp, sc):
            cols = slice(SC * sc, SC * (sc + 1))
            ps = pp.tile([128, SC], F32, tag="pp")
            for kc in range(4):
                nc.tensor.matmul(
                    ps,
                    lhsT=w_sb[:, kc, 128 * hp:128 * (hp + 1)],
                    rhs=xt[kc][:, cols],
                    start=(kc == 0), stop=(kc == 3),
                )
            pre = work.tile([128, SC], BF16, tag="pre")
            nc.scalar.copy(out=pre, in_=ps)
            psr = pp.tile([128, SC], F32, tag="pp")
            nc.tensor.matmul(psr, lhsT=p_sb, rhs=pre, start=True, stop=True)
            t1 = work.tile([128, SC], BF16, tag="t1")
            nc.vector.tensor_mul(out=t1, in0=pre, in1=cosF[:, cols])
            t2 = work.tile([128, SC], BF16, tag="t2")
            nc.vector.tensor_mul(out=t2, in0=psr, in1=sinF[:, cols])
            nc.gpsimd.tensor_add(out=dst[:, cols], in0=t1, in1=t2)

        # ---- v projection (natural layout, 65-col stride per head) ----
        def v_proj(vc):
            rows = slice(CK * vc, CK * (vc + 1))
            ps = pv.tile([CK, VW], F32, tag="pv")
            for kc in range(4):
                nc.tensor.matmul(
                    ps,
                    lhsT=xt[kc][:, rows],
                    rhs=wv_sb[:, kc, :],
                    start=(kc == 0), stop=(kc == 3),
                )
            # scale rows by attn_mask (ones in practice), cast to bf16
            nc.vector.tensor_scalar_mul(v_sb[vc], ps, am_sb[:, vc, 0:1])
            # write denominator column (attn_mask value) per head
            vv = v_sb[vc].rearrange("p (h e) -> p h e", e=D + 1)
            nc.vector.tensor_copy(out=vv[:, :, D], in_=am_sb[:, vc, :])

        # ---- attention ----
        exp_t = {}
        cs_t = {}

        def scores_chunk(j):
            qlo, qhi = _qwin(j)
            w = qhi - qlo
            for h in range(HPC):
                hp, hb = h // 2, 64 * (h % 2)
                ps = pss.tile([CK, MW], F32, tag="pss")
                nc.tensor.matmul(
                    ps[:, :w],
                    lhsT=kT[hp][hb:hb + 64, CK * j:CK * (j + 1)],
                    rhs=qT[hp][hb:hb + 64, qlo:qhi],
                    start=True, stop=True,
                )
                et = epool.tile([CK, MW], BF16, tag="exp")
                nc.scalar.activation(out=et[:, :w], in_=ps[:, :w],
                                     func=mybir.ActivationFunctionType.Exp,
                                     scale=0.125)
                mask = mf_sb if j == 0 else mi_sb[:, :w]
                # band-mask multiply: split across DVE / GPSIMD
                eng = nc.vector if h < 2 else nc.gpsimd
                eng.tensor_mul(out=et[:, :w], in0=et[:, :w], in1=mask)
                exp_t[(j, h)] = et

        def av_mm(i):
            # chunk i first: it covers the strip fully (start=True sets
            # has_written; the left neighbor accumulates on partitions 0:80)
            chunks = [c for c in (i, i - 1, i + 1) if 0 <= c < NCH]
            ps = pav.tile([CK, HPC, D + 1], F32, tag="pav")
            for h in range(HPC):
                for n, j in enumerate(chunks):
                    qlo, qhi = _qwin(j)
                    lo_g, hi_g = max(CK * i, qlo), min(CK * i + CK, qhi)
                    nc.tensor.matmul(
                        ps[0:hi_g - lo_g, h, :],
                        lhsT=exp_t[(j, h)][:, lo_g - qlo:hi_g - qlo],
                        rhs=v_sb[j][:, (D + 1) * h:(D + 1) * (h + 1)],
                        start=(n == 0), stop=(n == len(chunks) - 1),
                    )
            # per-query softmax normalization: reciprocal of the denominator
            # column, one broadcast-AP multiply for all 4 heads
            rcp = work.tile([CK, HPC], F32, tag="rcp")
            nc.vector.reciprocal(out=rcp, in_=ps[:, :, D])
            cs = work.tile([CK, HPC, D], BF16, tag="cs")
            nc.vector.tensor_mul(
                out=cs, in0=ps[:, :, 0:D],
                in1=rcp.unsqueeze(2).broadcast_to([CK, HPC, D]))
            cs_t[i] = cs

        def av_tr(i):
            # transpose [100 q, 128 hd-pair] -> ctxT [128, 100] per pair
            csf = cs_t.pop(i).rearrange("p h e -> p (h e)")
            for hp in range(2):
                pt = ptr.tile([128, CK], BF16, tag="ptr")
                nc.tensor.transpose(pt, csf[:, 128 * hp:128 * (hp + 1)], id_sb)
                nc.vector.tensor_copy(out=ctxT[hp][:, CK * i:CK * (i + 1)],
                                      in_=pt)

        # ---- output projection, one 128-row column block at a time ----
        def out_oc(c, oc):
            cols = slice(SC * c, SC * (c + 1))
            ps = pp.tile([128, SC], F32, tag="pp")
            for hp in range(2):
                nc.tensor.matmul(
                    ps,
                    lhsT=wo_sb[:, hp, 128 * oc:128 * (oc + 1)],
                    rhs=ctxT[hp][:, cols],
                    start=(hp == 0), stop=(hp == 1),
                )
            ost = work.tile([128, SC], BF16, tag="ost")
            if oc % 2 == 0:
                nc.scalar.copy(out=ost, in_=ps)
            else:
                nc.vector.tensor_copy(out=ost, in_=ps)
            nc.sync.dma_start(out=outT[128 * oc:128 * (oc + 1), cols], in_=ost)

        # ---- software-pipelined main loop ----
        # q/k projection block sc feeds score chunks [ranges[sc], ranges[sc+1])
        # (chunk j's query window ends at patch 5j+9 <= 25(sc+1)); the next
        # group's 4 projection calls are spread one per chunk so the PE gets
        # a uniform stream of dense N=500 matmuls (keeps the HAM un-throttled)
        def qk_call(sc, m):
            hp = m // 2
            if m % 2 == 0:
                qk_proj(wq_sb, qT[hp], hp, sc)
            else:
                qk_proj(wk_sb, kT[hp], hp, sc)

        ranges = [0, 4, 9, 14, NCH]
        for m in range(4):
            qk_call(0, m)
        for sc in range(NSC):
            for idx, j in enumerate(range(ranges[sc], ranges[sc + 1])):
                if sc + 1 < NSC and idx < 4:
                    qk_call(sc + 1, idx)
                v_proj(j)
                scores_chunk(j)
                if j >= 2:
                    av_tr(j - 2)
                if j >= 1:
                    av_mm(j - 1)
                # spread output projection: block c ready after av_tr(5c+4)
                if j >= 6 and (j - 6) % 5 < 4 and (j - 6) // 5 < 3:
                    out_oc((j - 6) // 5, (j - 6) % 5)
        av_mm(NCH - 1)
        av_tr(NCH - 2)
        av_tr(NCH - 1)
        for oc in range(4):
            out_oc(3, oc)

    nc.finalize()   # Bacc register allocation + DCE before serialization
    return nc


def _get_program():
    if "nc" not in _CACHE:
        _CACHE["nc"] = _build_program()
    return _CACHE["nc"]


def _host_prep(x, attn_mask, timestamps, Wq, Wk, Wv, Wo):
    """Build the 8 per-core input maps."""
    bf16 = ml_dtypes.bfloat16

    def to_tm(a):
        # [B, S, ...] space-major -> time-major (u = t*SP + sp)
        return (a.reshape(B, SP, T, *a.shape[2:])
                 .swapaxes(1, 2)
                 .reshape(B, S, *a.shape[2:]))

    x_tm = to_tm(np.ascontiguousarray(x))
    ts_tm = to_tm(np.ascontiguousarray(timestamps))
    amask_tm = to_tm(np.ascontiguousarray(attn_mask)).astype(np.float32)

    # the device program bakes the time-patch structure into its band masks
    # and per-patch RoPE tables; the reference generates exactly this pattern
    assert np.array_equal(
        ts_tm, np.broadcast_to(np.repeat(np.arange(T, dtype=ts_tm.dtype), SP),
                               (B, S))), "unexpected timestamp pattern"

    # RoPE tables, per time patch (expanded to per-token on device)
    inv_freq = 1.0 / (ROPE_BASE ** (np.arange(0, D, 2, dtype=np.float32) / D))
    tt = np.arange(T, dtype=np.float32)
    freqs = tt[:, None] * inv_freq[None, :]
    emb = np.concatenate([freqs, freqs], axis=-1)      # [T, D]
    cos_t = np.cos(emb).astype(np.float32).T           # [64, T]
    sin_t = np.sin(emb).astype(np.float32).T

    # rotation matrix (sign-carrying rotate-half), block-diag per head pair
    p = np.zeros((128, 128), np.float32)
    for blk in (0, 64):
        for d in range(32):
            p[blk + d + 32, blk + d] = -1.0
            p[blk + d, blk + d + 32] = 1.0

    # band masks (interior window starts at patch 5j-5; first at 0)
    kr = np.arange(CK)[:, None] // SP          # key patch within chunk [0,5)
    dlt = np.arange(MW)[None, :] // SP - kr
    m_int = ((dlt >= 1) & (dlt <= 9)).astype(np.float32)
    dlt0 = np.arange(180)[None, :] // SP - kr
    m_first = ((dlt0 >= -4) & (dlt0 <= 4)).astype(np.float32)

    in_maps = []
    for c in range(N_CORES):
        b, g = c // 2, c % 2
        hcols = slice(HG * g, HG * (g + 1))

        wv_ext = np.zeros((H, VW), np.float32)
        for h in range(HPC):
            wv_ext[:, (D + 1) * h:(D + 1) * h + D] = \
                Wv[:, HG * g + D * h:HG * g + D * (h + 1)]

        in_maps.append({
            "xT": np.ascontiguousarray(x_tm[b].T).astype(bf16),
            "wq": np.ascontiguousarray(Wq[:, hcols]).astype(bf16),
            "wk": np.ascontiguousarray(Wk[:, hcols]).astype(bf16),
            "wv": wv_ext.astype(bf16),
            "wo": np.ascontiguousarray(Wo[hcols, :]).astype(bf16),
            "cosT": np.vstack([cos_t, cos_t]).astype(bf16),
            "sinT": np.vstack([sin_t, sin_t]).astype(bf16),
            "p128": p.astype(bf16),
            "ident": np.eye(CK, dtype=np.float32).astype(bf16),
            "m_int": m_int.astype(bf16),
            "m_first": m_first.astype(bf16),
            "am": np.ascontiguousarray(
                np.repeat(amask_tm[b].reshape(NCH, CK, 1), HPC, axis=2)),
        })
    return in_maps


def kernel(x, attn_mask, timestamps, Wq, bq, Wk, bk, Wv, bv, Wo, bo,
           **_ignored):
    x = np.asarray(x, np.float32)
    attn_mask = np.asarray(attn_mask)
    timestamps = np.asarray(timestamps)
    Wq, Wk, Wv, Wo = (np.asarray(a, np.float32) for a in (Wq, Wk, Wv, Wo))
    bq, bk, bv, bo = (np.asarray(a, np.float32) for a in (bq, bk, bv, bo))
    assert not (np.any(bq) or np.any(bk) or np.any(bv)), \
        "nonzero qkv biases not supported"

    nc = _get_program()
    in_maps = _host_prep(x, attn_mask, timestamps, Wq, Wk, Wv, Wo)

    res = bass_utils.run_bass_kernel_spmd(nc, in_maps,
                                          core_ids=list(range(N_CORES)))
    _CACHE["last_results"] = res

    out = np.empty((B, S, H), np.float32)
    for b in range(B):
        o = (res.results[2 * b]["outT"].astype(np.float32) +
             res.results[2 * b + 1]["outT"].astype(np.float32))
        o_tm = o.T + bo[None, :]                        # [2000, 512]
        out[b] = (o_tm.reshape(T, SP, H)
                      .swapaxes(0, 1)
                      .reshape(S, H))
    return out
